# revision 12
# baseline (speedup 1.0000x reference)
import os
import sys

sys.path.insert(0, "/opt/trn_rl_repo")

import numpy as np
import ml_dtypes

import concourse.bass as bass
from concourse import bacc, mybir
from concourse.bass_utils import run_bass_kernel_spmd
from concourse.tile import TileContext

BF = ml_dtypes.bfloat16
F32 = mybir.dt.float32
BF16 = mybir.dt.bfloat16
AF = mybir.ActivationFunctionType
OP = mybir.AluOpType

B, T, IDIM, HDIM = 128, 256, 64, 128
OC1 = 100
NCORES = 8
BP = B // NCORES  # 16 rows per core
S1 = 58
S = S1 * S1       # 3364
SPAD = 3456       # 27*128, padded for DMA transpose
NCH = 27          # S chunks: 26x128 + 36
HN = 2 * HDIM     # 256
F = S + HN        # 3620
HID = F // 2      # 1810
ANF = 64

# K-chunks of F (for fc1 / attention-pre): 26x128, 36, then h0f(128), h1f(128)
FCH = [(i * 128, 128) for i in range(26)] + [(3328, 36), (3364, 128), (3492, 128)]
# M-chunks of HID
MCH = [(i * 128, 128) for i in range(14)] + [(1792, 18)]

_cache = {}


def _build():
    nc = bacc.Bacc("TRN2", target_bir_lowering=False, debug=False)

    # ---------------- DRAM I/O ----------------
    x27 = nc.dram_tensor("x27", [BP, 27, 62, 64], BF16, kind="ExternalInput").ap()
    x2T = nc.dram_tensor("x2T", [65, T * BP], BF16, kind="ExternalInput").ap()
    w1T = nc.dram_tensor("w1T", [27, OC1], BF16, kind="ExternalInput").ap()
    b1 = nc.dram_tensor("b1", [OC1, 1], F32, kind="ExternalInput").ap()
    w2a = nc.dram_tensor("w2a", [OC1, 9 * OC1], BF16, kind="ExternalInput").ap()
    b2a = nc.dram_tensor("b2a", [OC1, 1], F32, kind="ExternalInput").ap()
    w2b = nc.dram_tensor("w2b", [OC1, 9 * OC1], BF16, kind="ExternalInput").ap()
    b2b = nc.dram_tensor("b2b", [OC1, 1], F32, kind="ExternalInput").ap()
    wih0 = nc.dram_tensor("wih0", [65, 512], BF16, kind="ExternalInput").ap()
    whh0 = nc.dram_tensor("whh0", [128, 512], BF16, kind="ExternalInput").ap()
    wih1 = nc.dram_tensor("wih1", [128, 512], BF16, kind="ExternalInput").ap()
    whh1 = nc.dram_tensor("whh1", [128, 512], BF16, kind="ExternalInput").ap()
    bias1 = nc.dram_tensor("bias1", [128, 4], F32, kind="ExternalInput").ap()
    awST = nc.dram_tensor("awST", [S, ANF], BF16, kind="ExternalInput").ap()
    awHT = nc.dram_tensor("awHT", [HN, ANF], BF16, kind="ExternalInput").ap()
    ab1 = nc.dram_tensor("ab1", [ANF, 1], F32, kind="ExternalInput").ap()
    aw2T = nc.dram_tensor("aw2T", [ANF, 1], BF16, kind="ExternalInput").ap()
    fwT = nc.dram_tensor("fwT", [F, HID], BF16, kind="ExternalInput").ap()
    fb1p = nc.dram_tensor("fb1p", [128, 15], F32, kind="ExternalInput").ap()
    fw2p = nc.dram_tensor("fw2p", [128, 15], BF16, kind="ExternalInput").ap()
    fc2b = nc.dram_tensor("fc2b", [BP, 1], F32, kind="ExternalInput").ap()
    xdd = nc.dram_tensor("xdd", [BP, OC1, S], BF16, kind="Internal").ap()
    out = nc.dram_tensor("out", [BP, 1], F32, kind="ExternalOutput").ap()

    with TileContext(nc) as tc:
        with (
            tc.tile_pool(name="consts", bufs=1) as consts,
            tc.tile_pool(name="persist", bufs=1) as persist,
        ):
            # ---- load constants ----
            w1T_t = consts.tile([27, OC1], BF16)
            nc.sync.dma_start(out=w1T_t, in_=w1T)
            b1_t = consts.tile([OC1, 1], F32)
            nc.sync.dma_start(out=b1_t, in_=b1)
            w2a_t = consts.tile([OC1, 9 * OC1], BF16)
            nc.sync.dma_start(out=w2a_t, in_=w2a)
            b2a_t = consts.tile([OC1, 1], F32)
            nc.sync.dma_start(out=b2a_t, in_=b2a)
            w2b_t = consts.tile([OC1, 9 * OC1], BF16)
            nc.sync.dma_start(out=w2b_t, in_=w2b)
            b2b_t = consts.tile([OC1, 1], F32)
            nc.sync.dma_start(out=b2b_t, in_=b2b)
            wih0_t = consts.tile([65, 512], BF16)
            nc.sync.dma_start(out=wih0_t, in_=wih0)
            whh0_t = consts.tile([128, 512], BF16)
            nc.sync.dma_start(out=whh0_t, in_=whh0)
            wih1_t = consts.tile([128, 512], BF16)
            nc.sync.dma_start(out=wih1_t, in_=wih1)
            whh1_t = consts.tile([128, 512], BF16)
            nc.sync.dma_start(out=whh1_t, in_=whh1)
            bias1_t = consts.tile([128, 4], F32)
            nc.sync.dma_start(out=bias1_t, in_=bias1)
            awST_t = consts.tile([128, NCH, ANF], BF16)
            nc.sync.dma_start(
                out=awST_t[:, 0:26, :],
                in_=awST[0 : 26 * 128, :].rearrange("(c p) f -> p c f", p=128),
            )
            nc.sync.dma_start(out=awST_t[0:36, 26, :], in_=awST[3328:3364, :])
            awHT_t = consts.tile([128, 2, ANF], BF16)
            nc.sync.dma_start(
                out=awHT_t, in_=awHT.rearrange("(c p) f -> p c f", p=128)
            )
            ab1_t = consts.tile([ANF, 1], F32)
            nc.sync.dma_start(out=ab1_t, in_=ab1)
            aw2T_t = consts.tile([ANF, 1], BF16)
            nc.sync.dma_start(out=aw2T_t, in_=aw2T)
            fb1p_t = consts.tile([128, 15], F32)
            nc.sync.dma_start(out=fb1p_t, in_=fb1p)
            fw2p_t = consts.tile([128, 15], BF16)
            nc.sync.dma_start(out=fw2p_t, in_=fw2p)
            fc2b_t = consts.tile([BP, 1], F32)
            nc.sync.dma_start(out=fc2b_t, in_=fc2b)
            x2T_t = persist.tile([65, T * BP], BF16)
            nc.sync.dma_start(out=x2T_t, in_=x2T)

            ones100 = consts.tile([OC1, 1], BF16)
            nc.vector.memset(ones100, 1.0)
            ones1r = consts.tile([1, OC1], BF16)
            nc.vector.memset(ones1r, 1.0)
            zeroT = consts.tile([128, BP], BF16)
            nc.vector.memzero(zeroT)

            # ---- persistent state ----
            y0T_t = persist.tile([128, T * BP], BF16)   # layer0 outputs h0_t
            c0_t = persist.tile([128, BP], F32)
            nc.vector.memzero(c0_t)
            c1_t = persist.tile([128, BP], F32)
            nc.vector.memzero(c1_t)
            h1a = persist.tile([128, BP], BF16)
            h1b = persist.tile([128, BP], BF16)
            preS_t = persist.tile([ANF, BP * OC1], F32)  # attn pre (xd part)
            xd_a = persist.tile([128, SPAD], BF16)
            nc.vector.memzero(xd_a)
            xd_b = persist.tile([128, SPAD], BF16)
            nc.vector.memzero(xd_b)
            mT_t = persist.tile([128, NCH, BP], BF16)    # ctx^T chunks
            h1T_t = persist.tile([128, 15, BP], BF16)    # fc1 out chunks
            E_t = persist.tile([OC1, BP], BF16)
            aw_t = persist.tile([OC1, BP], BF16)
            rz_t = persist.tile([1, BP], BF16)
            preHb_t = persist.tile([ANF, BP], F32)
            out_t = persist.tile([BP, 1], F32)

            with (
                tc.tile_pool(name="cio", bufs=2) as cio,
                tc.tile_pool(name="cmid", bufs=2) as cmid,
                tc.tile_pool(name="cps", bufs=2, space="PSUM") as cps,
                tc.tile_pool(name="gps", bufs=2, space="PSUM") as gps,
                tc.tile_pool(name="pps", bufs=1, space="PSUM") as pps,
                tc.tile_pool(name="lsg", bufs=2) as lsg,
            ):

                def conv_gen():
                    for b in range(BP):
                        x27_t = cio.tile([27, 62, 64], BF16, tag="x27t", name="x27t")
                        nc.sync.dma_start(out=x27_t, in_=x27[b])
                        yield
                        # conv1 -> a1 [100, 62, 62]
                        a1_t = cmid.tile([OC1, 62, 62], BF16, tag="a1", name="a1")
                        r = 0
                        while r < 62:
                            rows = min(8, 62 - r)
                            ps = cps.tile([OC1, 496], F32, tag="cpsa", name="cpsa")
                            pv = ps[:, : rows * 62].rearrange(
                                "p (r c) -> p r c", r=rows
                            )
                            nc.tensor.matmul(
                                pv, w1T_t, x27_t[:, r : r + rows, 0:62],
                                start=True, stop=True,
                            )
                            yield
                            nc.scalar.activation(
                                out=a1_t[:, r : r + rows, :], in_=pv,
                                func=AF.Relu, bias=b1_t, scale=1.0,
                            )
                            yield
                            r += rows
                        # conv2a -> a2 [100, 60, 60]
                        a2_t = cmid.tile([OC1, 60, 60], BF16, tag="a2", name="a2")
                        r = 0
                        while r < 60:
                            rows = min(8, 60 - r)
                            ps = cps.tile([OC1, 496], F32, tag="cpsa", name="cpsa")
                            pv = ps[:, : rows * 60].rearrange(
                                "p (r c) -> p r c", r=rows
                            )
                            for tap in range(9):
                                ky, kx = divmod(tap, 3)
                                nc.tensor.matmul(
                                    pv,
                                    w2a_t[:, OC1 * tap : OC1 * (tap + 1)],
                                    a1_t[:, r + ky : r + ky + rows, kx : kx + 60],
                                    start=(tap == 0), stop=(tap == 8),
                                )
                                yield
                            nc.scalar.activation(
                                out=a2_t[:, r : r + rows, :], in_=pv,
                                func=AF.Relu, bias=b2a_t, scale=1.0,
                            )
                            yield
                            r += rows
                        # conv2b -> xd_t [128, 3456] (valid [0:100, 0:3364])
                        xd_t = xd_a if b % 2 == 0 else xd_b
                        r = 0
                        while r < S1:
                            rows = min(8, S1 - r)
                            ps = cps.tile([OC1, 496], F32, tag="cpsa", name="cpsa")
                            pv = ps[:, : rows * S1].rearrange(
                                "p (r c) -> p r c", r=rows
                            )
                            for tap in range(9):
                                ky, kx = divmod(tap, 3)
                                nc.tensor.matmul(
                                    pv,
                                    w2b_t[:, OC1 * tap : OC1 * (tap + 1)],
                                    a2_t[:, r + ky : r + ky + rows, kx : kx + S1],
                                    start=(tap == 0), stop=(tap == 8),
                                )
                                yield
                            nc.scalar.activation(
                                out=xd_t[0:OC1, r * S1 : (r + rows) * S1],
                                in_=ps[:, : rows * S1],
                                func=AF.Relu, bias=b2b_t, scale=1.0,
                            )
                            yield
                            r += rows
                        nc.sync.dma_start(out=xdd[b], in_=xd_t[0:OC1, 0:S])
                        yield
                        # transpose full image: xdT[p, c, f] = xd[f, 128c+p]
                        xdT_t = cio.tile([128, NCH, 128], BF16, tag="xdT", name="xdT")
                        nc.sync.dma_start_transpose(out=xdT_t, in_=xd_t)
                        yield
                        # attn pre (xd part): preS[:, b*100:(b+1)*100]
                        pre_ps = pps.tile([ANF, OC1], F32, tag="preps", name="preps")
                        for c in range(NCH):
                            kw = 128 if c < 26 else 36
                            nc.tensor.matmul(
                                pre_ps,
                                awST_t[0:kw, c, :],
                                xdT_t[0:kw, c, 0:OC1],
                                start=(c == 0), stop=(c == NCH - 1),
                            )
                            yield
                        nc.scalar.activation(
                            out=preS_t[:, b * OC1 : (b + 1) * OC1], in_=pre_ps,
                            func=AF.Copy,
                        )
                        yield

                def lstm_step(layer, t):
                    if layer == 0:
                        g_ps = gps.tile([128, 64], F32, tag="g0", name="g0")
                        rhs_x = x2T_t[:, t * BP : (t + 1) * BP]
                        rhs_h = zeroT if t == 0 else y0T_t[:, (t - 1) * BP : t * BP]
                        wih, whh, c_t = wih0_t, whh0_t, c0_t
                    else:
                        g_ps = gps.tile([128, 64], F32, tag="g1", name="g1")
                        rhs_x = y0T_t[:, t * BP : (t + 1) * BP]
                        if t == 0:
                            rhs_h = zeroT
                        else:
                            rhs_h = h1a if (t - 1) % 2 == 0 else h1b
                        wih, whh, c_t = wih1_t, whh1_t, c1_t
                    for q in range(4):
                        nc.tensor.matmul(
                            g_ps[:, q * BP : (q + 1) * BP],
                            wih[:, 128 * q : 128 * (q + 1)],
                            rhs_x, start=True, stop=False,
                        )
                        nc.tensor.matmul(
                            g_ps[:, q * BP : (q + 1) * BP],
                            whh[:, 128 * q : 128 * (q + 1)],
                            rhs_h, start=False, stop=True,
                        )
                    sg = lsg.tile([128, 64], F32, tag=f"sg{layer}", name="sg")
                    if layer == 0:
                        nc.scalar.activation(
                            out=sg[:, 0:48], in_=g_ps[:, 0:48], func=AF.Sigmoid
                        )
                        nc.scalar.activation(
                            out=sg[:, 48:64], in_=g_ps[:, 48:64], func=AF.Tanh
                        )
                    else:
                        for q in range(4):
                            nc.scalar.activation(
                                out=sg[:, q * BP : (q + 1) * BP],
                                in_=g_ps[:, q * BP : (q + 1) * BP],
                                func=AF.Sigmoid if q < 3 else AF.Tanh,
                                bias=bias1_t[:, q : q + 1],
                            )
                    t1 = lsg.tile([128, BP], F32, tag=f"t1_{layer}", name="t1")
                    t2 = lsg.tile([128, BP], F32, tag=f"t2_{layer}", name="t2")
                    nc.vector.tensor_tensor(t1, sg[:, BP : 2 * BP], c_t, op=OP.mult)
                    nc.vector.tensor_tensor(
                        t2, sg[:, 0:BP], sg[:, 3 * BP : 4 * BP], op=OP.mult
                    )
                    nc.vector.tensor_tensor(c_t, t1, t2, op=OP.add)
                    tch = lsg.tile([128, BP], F32, tag=f"tc_{layer}", name="tch")
                    nc.scalar.activation(out=tch, in_=c_t, func=AF.Tanh)
                    if layer == 0:
                        h_out = y0T_t[:, t * BP : (t + 1) * BP]
                    else:
                        h_out = h1a if t % 2 == 0 else h1b
                    nc.vector.tensor_tensor(
                        h_out, sg[:, 2 * BP : 3 * BP], tch, op=OP.mult
                    )

                # ---- interleaved main phase ----
                cg = conv_gen()
                conv_done = False
                for t in range(T + 1):
                    for _ in range(13):
                        if conv_done:
                            break
                        if next(cg, "done") == "done":
                            conv_done = True
                    if t < T:
                        lstm_step(0, t)
                    if t >= 1:
                        lstm_step(1, t - 1)
                while not conv_done:
                    if next(cg, "done") == "done":
                        conv_done = True

                # ---- attention hn part ----
                h0fT = y0T_t[:, (T - 1) * BP : T * BP]
                h1fT = h1b  # t=255 odd
                ph_ps = pps.tile([ANF, BP], F32, tag="phn", name="phn")
                nc.tensor.matmul(ph_ps, awHT_t[:, 0, :], h0fT, start=True, stop=False)
                nc.tensor.matmul(ph_ps, awHT_t[:, 1, :], h1fT, start=False, stop=True)
                nc.vector.tensor_scalar_add(preHb_t, ph_ps, ab1_t)

            # conv/lstm psum pools released here
            with (
                tc.tile_pool(name="aps", bufs=2, space="PSUM") as aps,
                tc.tile_pool(name="fps", bufs=1, space="PSUM") as fps,
                tc.tile_pool(name="aio", bufs=2) as aio,
                tc.tile_pool(name="fwall", bufs=1) as fwall,
            ):
                # stream in all fc1 weight chunks (resident for mc-outer loop)
                fw_ts = []
                for kc, (off, kw) in enumerate(FCH):
                    fw_t = fwall.tile(
                        [128, HID], BF16, tag=f"fw{kc}", name=f"fw{kc}"
                    )
                    nc.sync.dma_start(out=fw_t[0:kw, :], in_=fwT[off : off + kw, :])
                    fw_ts.append(fw_t)
                # scores for all images
                s_ps = fps.tile([OC1, BP], F32, tag="sps", name="sps")
                for b in range(BP):
                    aT = aio.tile([ANF, OC1], BF16, tag="aT", name="aT")
                    nc.scalar.activation(
                        out=aT, in_=preS_t[:, b * OC1 : (b + 1) * OC1],
                        func=AF.Tanh, bias=preHb_t[:, b : b + 1],
                    )
                    nc.tensor.matmul(
                        s_ps[:, b : b + 1], aT, aw2T_t, start=True, stop=True
                    )
                nc.scalar.activation(out=E_t, in_=s_ps, func=AF.Exp)
                z_ps = fps.tile([1, BP], F32, tag="zps", name="zps")
                nc.tensor.matmul(z_ps, ones100, E_t, start=True, stop=True)
                rzf_t = persist.tile([1, BP], F32, name="rzf_t")
                nc.vector.reciprocal(rzf_t, z_ps)
                nc.vector.tensor_copy(rz_t, rzf_t)
                rzb_ps = fps.tile([OC1, BP], F32, tag="rzb", name="rzb")
                nc.tensor.matmul(rzb_ps, ones1r, rz_t, start=True, stop=True)
                nc.vector.tensor_tensor(aw_t, E_t, rzb_ps, op=OP.mult)

                # ctx chunks: mT[:, c, :] = sum_ch xd[ch, chunk_c] * aw[ch, b]
                for c in range(NCH):
                    off, kw = FCH[c]
                    xdc_t = aio.tile([OC1, BP, 128], BF16, tag="xdc", name="xdc")
                    nc.sync.dma_start(
                        out=xdc_t[:, :, 0:kw],
                        in_=xdd[:, :, off : off + kw].rearrange("b p f -> p b f"),
                    )
                    ctx_ps = aps.tile([128, BP], F32, tag="ctxps", name="ctxps")
                    for b in range(BP):
                        nc.tensor.matmul(
                            ctx_ps[0:kw, b : b + 1],
                            xdc_t[:, b, 0:kw],
                            aw_t[:, b : b + 1],
                            start=True, stop=True,
                        )
                    nc.scalar.activation(
                        out=mT_t[0:kw, c, :], in_=ctx_ps[0:kw, :], func=AF.Copy
                    )

                # fc1: h1T = relu(fc1_w @ m + b); one psum group at a time
                rhs_chunks = [mT_t[:, c, :] for c in range(NCH)] + [h0fT, h1fT]
                h1_ps = fps.tile([128, 15, BP], F32, tag="h1ps", name="h1ps")
                for mc, (moff, mw) in enumerate(MCH):
                    for kc, (off, kw) in enumerate(FCH):
                        nc.tensor.matmul(
                            h1_ps[0:mw, mc, :],
                            fw_ts[kc][0:kw, moff : moff + mw],
                            rhs_chunks[kc][0:kw, :],
                            start=(kc == 0), stop=(kc == 28),
                        )
                    nc.scalar.activation(
                        out=h1T_t[0:mw, mc, :], in_=h1_ps[0:mw, mc, :],
                        func=AF.Relu, bias=fb1p_t[0:mw, mc : mc + 1],
                    )
                # fc2
                o_ps = fps.tile([BP, 1], F32, tag="ops", name="ops")
                for mc, (moff, mw) in enumerate(MCH):
                    nc.tensor.matmul(
                        o_ps,
                        h1T_t[0:mw, mc, :],
                        fw2p_t[0:mw, mc : mc + 1],
                        start=(mc == 0), stop=(mc == 14),
                    )
                nc.scalar.activation(out=out_t, in_=o_ps, func=AF.Identity, bias=fc2b_t)
                nc.sync.dma_start(out=out, in_=out_t)

    nc.compile()
    return nc


def _prep_shared(conv1_w, conv1_b, conv2a_w, conv2a_b, conv2b_w, conv2b_b,
                 w_ih0, w_hh0, b_ih0, b_hh0, w_ih1, w_hh1, b_ih1, b_hh1,
                 attn1_w, attn1_b, attn2_w, attn2_b, fc1_w, fc1_b, fc2_w, fc2_b):
    perm = np.concatenate([
        np.arange(0, 128), np.arange(128, 256),
        np.arange(384, 512), np.arange(256, 384),
    ])
    sh = {}
    sh["w1T"] = np.ascontiguousarray(
        conv1_w.transpose(2, 3, 1, 0).reshape(27, OC1)).astype(BF)
    sh["b1"] = conv1_b.reshape(OC1, 1).astype(np.float32)
    sh["w2a"] = np.ascontiguousarray(
        conv2a_w.transpose(1, 2, 3, 0).reshape(OC1, 900)).astype(BF)
    sh["b2a"] = conv2a_b.reshape(OC1, 1).astype(np.float32)
    sh["w2b"] = np.ascontiguousarray(
        conv2b_w.transpose(1, 2, 3, 0).reshape(OC1, 900)).astype(BF)
    sh["b2b"] = conv2b_b.reshape(OC1, 1).astype(np.float32)
    wih0t = w_ih0[perm].T.astype(np.float32)              # [64, 512]
    bias0 = (b_ih0 + b_hh0)[perm].astype(np.float32)      # [512]
    sh["wih0"] = np.concatenate([wih0t, bias0[None, :]], axis=0).astype(BF)
    sh["whh0"] = np.ascontiguousarray(w_hh0[perm].T).astype(BF)
    sh["wih1"] = np.ascontiguousarray(w_ih1[perm].T).astype(BF)
    sh["whh1"] = np.ascontiguousarray(w_hh1[perm].T).astype(BF)
    sh["bias1"] = np.ascontiguousarray(
        (b_ih1 + b_hh1)[perm].reshape(4, 128).T).astype(np.float32)
    sh["awST"] = np.ascontiguousarray(attn1_w[:, :S].T).astype(BF)
    sh["awHT"] = np.ascontiguousarray(attn1_w[:, S:].T).astype(BF)
    sh["ab1"] = attn1_b.reshape(ANF, 1).astype(np.float32)
    sh["aw2T"] = attn2_w.reshape(1, ANF).T.astype(BF)
    sh["fwT"] = np.ascontiguousarray(fc1_w.T).astype(BF)
    fb1p = np.zeros((15, 128), np.float32)
    fb1p.ravel()[:HID] = fc1_b
    sh["fb1p"] = np.ascontiguousarray(fb1p.T)
    fw2p = np.zeros((15, 128), np.float32)
    fw2p.ravel()[:HID] = fc2_w[0]
    sh["fw2p"] = np.ascontiguousarray(fw2p.T).astype(BF)
    sh["fc2b"] = np.full((BP, 1), float(fc2_b[0]), np.float32)
    return sh


def _prep_core(x1s, x2s):
    # x27[b, (ky,kx,c), y, x] = x1[b, c, y+ky, x+kx]
    x27 = np.zeros((BP, 9, 3, 62, 64), np.float32)
    for ky in range(3):
        for kx in range(3):
            x27[:, ky * 3 + kx, :, :, 0:62] = x1s[:, :, ky : ky + 62, kx : kx + 62]
    x2T = np.concatenate(
        [
            x2s.transpose(2, 1, 0).reshape(IDIM, T * BP),
            np.ones((1, T * BP), np.float32),
        ],
        axis=0,
    )
    return {
        "x27": x27.reshape(BP, 27, 62, 64).astype(BF),
        "x2T": x2T.astype(BF),
    }


def kernel(x1, x2, conv1_w, conv1_b, conv2a_w, conv2a_b, conv2b_w, conv2b_b,
           w_ih0, w_hh0, b_ih0, b_hh0, w_ih1, w_hh1, b_ih1, b_hh1,
           attn1_w, attn1_b, attn2_w, attn2_b, fc1_w, fc1_b, fc2_w, fc2_b):
    if "nc" not in _cache:
        _cache["nc"] = _build()
    nc = _cache["nc"]

    sh = _prep_shared(conv1_w, conv1_b, conv2a_w, conv2a_b, conv2b_w, conv2b_b,
                      w_ih0, w_hh0, b_ih0, b_hh0, w_ih1, w_hh1, b_ih1, b_hh1,
                      attn1_w, attn1_b, attn2_w, attn2_b, fc1_w, fc1_b,
                      fc2_w, fc2_b)
    in_maps = []
    for c in range(NCORES):
        m = dict(sh)
        m.update(_prep_core(
            np.asarray(x1[c * BP : (c + 1) * BP], np.float32),
            np.asarray(x2[c * BP : (c + 1) * BP], np.float32),
        ))
        in_maps.append(m)

    tracedir = os.environ.get("KTRACE_DIR") or None
    if tracedir:
        os.makedirs(tracedir, exist_ok=True)
    res = run_bass_kernel_spmd(
        nc, in_maps, core_ids=list(range(NCORES)), tmpdir=tracedir
    )
    _cache["last_results"] = res
    out = np.concatenate(
        [np.asarray(res.results[i]["out"], np.float32) for i in range(NCORES)],
        axis=0,
    )
    return out
